# revision 9
# baseline (speedup 1.0000x reference)
"""Cross-attention (B=2, T=2048, E=1024, H=16, Dh=64) on 8 trn2 NeuronCores.

Sharding: core c = b*4 + g  ->  batch b in {0,1}, head-group g in {0..3}
(4 heads per core).  Each core computes its heads' Q/K/V projections,
attention, and a partial out-projection; the host sums the 4 head-group
partials per batch (the tensor-parallel all-reduce, done at unshard
time) and adds the bias correction  bo + Wo @ bv.

Layout strategy: activations are fed to the device pre-transposed
([E, T] instead of [T, E]) so every matmul operand has its contraction
dim on partitions with no on-chip transposes.  Weights are fed as
Wq/Wk/Wv row-slices transposed ([E, 256]) and the Wo column-slice
transposed ([256, E]).

Attention math per head (no max-subtraction needed: |scores| <~ 10):
  S^T[k,q]  = K @ Q^T                (lhsT = K^T tile, rhs = Q^T tile;
              two heads share the PE array via row tiling at
              partition bases 0 and 64)
  Ex        = exp(S^T / 8)           (ScalarE, scale folded into exp,
              1024-wide over two k-tiles to amortize op overhead)
  PV[d,q]   = sum_k V_aug[k,d] * Ex[k,q]   (V_aug has a ones column ->
              row 64 accumulates the softmax denominator)
  A^T[d,q]  = PV[0:64,q] * (1 / PV[64,q])  (denominator broadcast to 64
              partitions with a K=1 matmul against a ones row)
  out      += A^T.T @ WoT_slice      (partial; host sums over groups)
"""

import numpy as np

import concourse.bacc as bacc
import concourse.mybir as mybir
import concourse.tile as tile
from concourse.bass_utils import run_bass_kernel_spmd

E = 1024          # embed dim
T = 2048          # seq len (q and k)
DH = 64           # head dim
DLOC = 256        # per-core projected dim (4 heads * 64)
ET = E // 128     # 8 contraction tiles over embed
KT = T // 128     # 16 k-tiles
KTP = KT // 2     # 8 k-tile pairs (exp processes 1024-wide)
TB = T // 128     # 16 t-blocks
QB = T // 512     # 4 q-blocks of 512
PAIRS = 2         # head pairs per core (2 heads share the 128 partitions)
SCALE = float(1.0 / np.sqrt(DH))

F32 = mybir.dt.float32
# Matmul compute dtype: float32r streams 1 row/cycle (vs 4 for float32)
# when the moving free dim is >= 256.  Flip to F32 for full precision.
MM_DT = mybir.dt.float32r


CDT = MM_DT  # dtype for every tile that feeds a matmul


def build_nc():
    nc = bacc.Bacc("TRN2", target_bir_lowering=False, debug=False)

    xq_d = nc.dram_tensor("xq", [E, T], CDT, kind="ExternalInput")
    xk_d = nc.dram_tensor("xk", [E, T], CDT, kind="ExternalInput")
    xv_d = nc.dram_tensor("xv", [E, T], CDT, kind="ExternalInput")
    wq_d = nc.dram_tensor("wqt", [E, DLOC], CDT, kind="ExternalInput")
    wk_d = nc.dram_tensor("wkt", [E, DLOC], CDT, kind="ExternalInput")
    wv_d = nc.dram_tensor("wvt", [E, DLOC], CDT, kind="ExternalInput")
    wo_d = nc.dram_tensor("wot", [DLOC, E], CDT, kind="ExternalInput")
    bq_d = nc.dram_tensor("bq", [DLOC], F32, kind="ExternalInput")
    bk_d = nc.dram_tensor("bk", [DLOC], F32, kind="ExternalInput")
    ones_d = nc.dram_tensor("ones", [DH], CDT, kind="ExternalInput")
    out_d = nc.dram_tensor("out", [T, E], F32, kind="ExternalOutput")

    with tile.TileContext(nc) as tc:
        from contextlib import ExitStack

        with ExitStack() as ctx:
            persist = ctx.enter_context(tc.tile_pool(name="persist", bufs=1))
            wpool = ctx.enter_context(tc.tile_pool(name="wpool", bufs=1))
            xpool = ctx.enter_context(tc.tile_pool(name="xpool", bufs=2))
            epool = ctx.enter_context(tc.tile_pool(name="epool", bufs=4))
            spool = ctx.enter_context(tc.tile_pool(name="spool", bufs=2))
            opool = ctx.enter_context(tc.tile_pool(name="opool", bufs=2))
            # PSUM: 8 banks total.  s2 tiles are 2 banks each (1024 wide).
            psum2 = ctx.enter_context(
                tc.tile_pool(name="psum2", bufs=2, space="PSUM")
            )
            pvp = ctx.enter_context(
                tc.tile_pool(name="pvp", bufs=2, space="PSUM")
            )
            misc = ctx.enter_context(
                tc.tile_pool(name="miscp", bufs=2, space="PSUM")
            )

            # ---- persistent tiles ----
            qT = [
                persist.tile([128, T], CDT, name=f"qT{p}", tag=f"qT{p}")
                for p in range(PAIRS)
            ]
            kTt = [
                persist.tile([128, T], CDT, name=f"kT{p}", tag=f"kT{p}")
                for p in range(PAIRS)
            ]
            aT = [
                persist.tile([128, T], CDT, name=f"aT{p}", tag=f"aT{p}")
                for p in range(PAIRS)
            ]
            # V with a ones column per head: [t-part, t-block, head, 65]
            v_t = persist.tile([128, KT, 4, DH + 1], CDT, name="v_t", tag="v_t")
            ones_t = persist.tile([1, DH], CDT, name="ones_t", tag="ones_t")
            bq_t = persist.tile([128, 2], F32, name="bq_t", tag="bq_t")
            bk_t = persist.tile([128, 2], F32, name="bk_t", tag="bk_t")

            import concourse.bass as bass

            nc.sync.dma_start(
                out=ones_t[:],
                in_=ones_d.ap().rearrange("(a b) -> a b", a=1),
            )
            # ones column of v_t: partition-broadcast DMA of 64 ones
            ones_bcast = bass.AP(
                tensor=ones_d.ap().tensor,
                offset=0,
                ap=[[0, 128], [4, KT], [1, 4]],
            )
            nc.sync.dma_start(out=v_t[:, :, :, DH], in_=ones_bcast)

            # ---- weights / biases (emitted lazily, just before use, so
            #      the DMA queues prioritize the activation stream) ----
            wq_t = wpool.tile([128, ET, DLOC], CDT, name="wq_t", tag="wq")
            wk_t = wpool.tile([128, ET, DLOC], CDT, name="wk_t", tag="wk")
            wv_t = wpool.tile([128, ET, DLOC], CDT, name="wv_t", tag="wv")
            wo_t = wpool.tile([128, 2, E], CDT, name="wo_t", tag="wo")

            # ---- phase A: q/k projections -> qT, kT ([d_loc, t]) ----
            for nm, x_d, w_d, w_t, b_d, b_t, dst in (
                ("q", xq_d, wq_d, wq_t, bq_d, bq_t, qT),
                ("k", xk_d, wk_d, wk_t, bk_d, bk_t, kTt),
            ):
                nc.sync.dma_start(
                    out=w_t[:], in_=w_d.ap().rearrange("(n p) m -> p n m", p=128)
                )
                nc.sync.dma_start(
                    out=b_t[:], in_=b_d.ap().rearrange("(m p) -> p m", p=128)
                )
                x_r = x_d.ap().rearrange("(n p) t -> p n t", p=128)
                for half in range(2):
                    xh = xpool.tile(
                        [128, ET, T // 2], CDT, name=f"x_{nm}{half}", tag="x"
                    )
                    for et in range(ET):
                        nc.sync.dma_start(
                            out=xh[:, et, :],
                            in_=x_r[:, et, half * 1024 : (half + 1) * 1024],
                        )
                    for m in range(PAIRS):
                        for nb in range(2):
                            ps = misc.tile(
                                [128, 512],
                                F32,
                                name=f"ps_{nm}{half}{m}{nb}",
                                tag="ps",
                            )
                            for et in range(ET):
                                nc.tensor.matmul(
                                    ps[:],
                                    w_t[:, et, m * 128 : (m + 1) * 128],
                                    xh[:, et, nb * 512 : (nb + 1) * 512],
                                    start=(et == 0),
                                    stop=(et == ET - 1),
                                )
                            col = half * 1024 + nb * 512
                            nc.vector.tensor_scalar_add(
                                out=dst[m][:, col : col + 512],
                                in0=ps[:],
                                scalar1=b_t[:, m : m + 1],
                            )

            # ---- v projection -> v_t ([t, head, d] + ones col) ----
            nc.sync.dma_start(
                out=wv_t[:], in_=wv_d.ap().rearrange("(n p) m -> p n m", p=128)
            )
            nc.sync.dma_start(
                out=wo_t[:], in_=wo_d.ap().rearrange("(n p) m -> p n m", p=128)
            )
            xv_r = xv_d.ap().rearrange("(n p) t -> p n t", p=128)
            for half in range(2):
                xh = xpool.tile([128, ET, T // 2], CDT, name=f"x_v{half}", tag="x")
                for et in range(ET):
                    nc.sync.dma_start(
                        out=xh[:, et, :],
                        in_=xv_r[:, et, half * 1024 : (half + 1) * 1024],
                    )
                for tbl in range(TB // 2):
                    tb = half * (TB // 2) + tbl
                    ps = misc.tile([128, DLOC], F32, name=f"ps_v{tb}", tag="ps")
                    for et in range(ET):
                        nc.tensor.matmul(
                            ps[:],
                            xh[:, et, tbl * 128 : (tbl + 1) * 128],
                            wv_t[:, et, :],
                            start=(et == 0),
                            stop=(et == ET - 1),
                        )
                    nc.vector.tensor_copy(
                        out=v_t[:, tb, :, 0:DH],
                        in_=ps.rearrange("p (h d) -> p h d", h=4),
                    )

            # ---- phase B + C: attention per (q-block, pair), then the
            #      out-projection columns for that q-block ----
            def emit_s_exp(pair, qb, ktp):
                """S^T for k-tiles 2*ktp, 2*ktp+1 (both heads) + 1024-wide
                exp.  Returns the two exp tiles (head A, head B)."""
                qsl = slice(qb * 512, (qb + 1) * 512)
                ets = []
                for hh in range(2):
                    base = hh * DH
                    s2 = psum2.tile(
                        [128, 1024], F32, name=f"s{hh}_{pair}_{qb}_{ktp}", tag="s2"
                    )
                    for j in range(2):
                        kt = 2 * ktp + j
                        ksl = slice(kt * 128, (kt + 1) * 128)
                        nc.tensor.matmul(
                            s2[:, j * 512 : (j + 1) * 512],
                            kTt[pair][base : base + DH, ksl],
                            qT[pair][base : base + DH, qsl],
                            start=True,
                            stop=True,
                        )
                    e_t = epool.tile(
                        [128, 1024], CDT, name=f"e{hh}_{pair}_{qb}_{ktp}", tag="e"
                    )
                    nc.scalar.activation(
                        out=e_t[:],
                        in_=s2[:],
                        func=mybir.ActivationFunctionType.Exp,
                        bias=0.0,
                        scale=SCALE,
                    )
                    ets.append(e_t)
                return ets

            for qb in range(QB):
                qsl = slice(qb * 512, (qb + 1) * 512)
                for pair in range(PAIRS):
                    pv = [
                        pvp.tile(
                            [128, 512], F32, name=f"pv{hh}_{pair}_{qb}", tag="pv"
                        )
                        for hh in range(2)
                    ]
                    ets = emit_s_exp(pair, qb, 0)
                    for ktp in range(KTP):
                        cur, ets = ets, (
                            emit_s_exp(pair, qb, ktp + 1)
                            if ktp + 1 < KTP
                            else None
                        )
                        for hh in range(2):
                            for j in range(2):
                                kt = 2 * ktp + j
                                nc.tensor.matmul(
                                    pv[hh][0 : DH + 1, :],
                                    v_t[:, kt, 2 * pair + hh, :],
                                    cur[hh][:, j * 512 : (j + 1) * 512],
                                    start=(kt == 0),
                                    stop=(kt == KT - 1),
                                )
                    for hh in range(2):
                        recip = spool.tile(
                            [1, 512], CDT, name=f"rc{hh}_{pair}_{qb}", tag="recip"
                        )
                        with nc.allow_low_precision(
                            reason="f32r view; reciprocal keeps f32 bits"
                        ):
                            nc.vector.reciprocal(
                                out=recip[:], in_=pv[hh][DH : DH + 1, :]
                            )
                        bc = misc.tile(
                            [128, 512], F32, name=f"bc{hh}_{pair}_{qb}", tag="ps"
                        )
                        nc.tensor.matmul(
                            bc[0:DH, :],
                            ones_t[:],
                            recip[:],
                            start=True,
                            stop=True,
                        )
                        pvs = spool.tile(
                            [DH, 512], F32, name=f"pvs{hh}_{pair}_{qb}", tag="pvs"
                        )
                        nc.vector.tensor_copy(out=pvs[:], in_=pv[hh][0:DH, :])
                        nc.vector.tensor_mul(
                            out=aT[pair][hh * DH : (hh + 1) * DH, qsl],
                            in0=pvs[:],
                            in1=bc[0:DH, :],
                        )
                # out-projection for the 4 t-blocks covered by this q-block
                for tbl in range(4):
                    tb = qb * 4 + tbl
                    for eb in range(2):
                        ps = misc.tile(
                            [128, 512], F32, name=f"ps_o{tb}{eb}", tag="ps"
                        )
                        for kt2 in range(2):
                            nc.tensor.matmul(
                                ps[:],
                                aT[kt2][:, tb * 128 : (tb + 1) * 128],
                                wo_t[:, kt2, eb * 512 : (eb + 1) * 512],
                                start=(kt2 == 0),
                                stop=(kt2 == 1),
                            )
                        o_s = opool.tile(
                            [128, 512], F32, name=f"o_{tb}{eb}", tag="o"
                        )
                        nc.vector.tensor_copy(out=o_s[:], in_=ps[:])
                        nc.sync.dma_start(
                            out=out_d.ap()[
                                tb * 128 : (tb + 1) * 128,
                                eb * 512 : (eb + 1) * 512,
                            ],
                            in_=o_s[:],
                        )

    nc.compile()
    return nc


_NC = None


def get_nc():
    global _NC
    if _NC is None:
        _NC = build_nc()
    return _NC


def make_in_maps(query, key, value, Wq, bq, Wk, bk, Wv, bv, Wo, bo):
    query = np.asarray(query, dtype=np.float32)
    key = np.asarray(key, dtype=np.float32)
    value = np.asarray(value, dtype=np.float32)
    Wq = np.asarray(Wq, dtype=np.float32)
    Wk = np.asarray(Wk, dtype=np.float32)
    Wv = np.asarray(Wv, dtype=np.float32)
    Wo = np.asarray(Wo, dtype=np.float32)
    bq = np.asarray(bq, dtype=np.float32)
    bk = np.asarray(bk, dtype=np.float32)

    xq = [np.ascontiguousarray(query[b].T) for b in range(2)]
    xk = [np.ascontiguousarray(key[b].T) for b in range(2)]
    xv = [np.ascontiguousarray(value[b].T) for b in range(2)]
    wqt = [np.ascontiguousarray(Wq[g * DLOC : (g + 1) * DLOC, :].T) for g in range(4)]
    wkt = [np.ascontiguousarray(Wk[g * DLOC : (g + 1) * DLOC, :].T) for g in range(4)]
    wvt = [np.ascontiguousarray(Wv[g * DLOC : (g + 1) * DLOC, :].T) for g in range(4)]
    wot = [np.ascontiguousarray(Wo[:, g * DLOC : (g + 1) * DLOC].T) for g in range(4)]

    in_maps = []
    for c in range(8):
        b, g = divmod(c, 4)
        in_maps.append(
            {
                "xq": xq[b],
                "xk": xk[b],
                "xv": xv[b],
                "wqt": wqt[g],
                "wkt": wkt[g],
                "wvt": wvt[g],
                "wot": wot[g],
                "bq": np.ascontiguousarray(bq[g * DLOC : (g + 1) * DLOC]),
                "bk": np.ascontiguousarray(bk[g * DLOC : (g + 1) * DLOC]),
                "ones": np.ones(DH, dtype=np.float32),
            }
        )
    return in_maps


def kernel(query, key, value, Wq, bq, Wk, bk, Wv, bv, Wo, bo):
    in_maps = make_in_maps(query, key, value, Wq, bq, Wk, bk, Wv, bv, Wo, bo)
    nc = get_nc()
    res = run_bass_kernel_spmd(nc, in_maps, core_ids=list(range(8)))
    parts = [res.results[c]["out"] for c in range(8)]
    Wo_np = np.asarray(Wo, dtype=np.float32)
    bv_np = np.asarray(bv, dtype=np.float32)
    bo_np = np.asarray(bo, dtype=np.float32)
    corr = bo_np + Wo_np @ bv_np
    out = np.empty((2, T, E), dtype=np.float32)
    for b in range(2):
        acc = parts[b * 4].astype(np.float32)
        for g in range(1, 4):
            acc = acc + parts[b * 4 + g]
        out[b] = acc + corr[None, :]
    return out
